# revision 1
# baseline (speedup 1.0000x reference)
"""Trainium2 Bass kernel for BPPS model (LayerNorm -> per-species MLP -> segment sum).

Self-contained: hardcodes shapes from the problem spec.
  ps [200000, 512] f32, species_idx [200000] int, batch [200000] int (sorted),
  ln_gamma/ln_beta [512], W1 [4,512,256], W2 [4,256,256], W3 [4,256,1], W_comp [1,4].
Output: energies [2000, 1] f32.

Strategy: data-parallel over atoms on 8 NeuronCores; host work is layout only.
Device pipeline per 128-atom tile (fp8e4 + DoubleRow matmuls):
  L1 (2 DR matmuls, mean-subtraction folded into weights as a rank-1
  projection), Gram self-product for sum(x^2) with fused diag-extract
  (scalar_tensor_tensor), rs = 1/(sW1*sqrt(q/512+eps)) batched per quad,
  prescale z by rs (DVE/Pool), quad-batched scale-free Silu (ACT), uint16
  packed transpose -> L2 (1 DR matmul with interleaved W2), Silu, segment
  one-hot matmul (1 DR matmul per tile pair).
Per-core output: [16, 128, 257] f32 block sums (sum of h2 per structure bin +
atom counts); host applies W3 and the composition term.
"""

import sys

sys.path.insert(0, "/opt/trn_rl_repo")

import numpy as np
import ml_dtypes

BF16 = ml_dtypes.bfloat16
FP8 = ml_dtypes.float8_e4m3

# Problem constants
N_ATOMS = 200000
D_IN = 512
HIDDEN = 256
N_SPECIES = 4
N_STRUCT = 2000
AVG_N_ATOMS = 60.0
E_SCALE = 1.0
LN_EPS = 1e-5

N_CORES = 8
ATOMS_PER_CORE = 25088          # 8 * 25088 = 200704 >= 200000
N_GROUPS = 4                    # groups per core
GROUP_ATOMS = ATOMS_PER_CORE // N_GROUPS   # 6272
BINS = 128
P = 128
FP8_MAX = 224.0                 # headroom under trn e4m3 max (240)


def _q8(x):
    return np.clip(x, -FP8_MAX, FP8_MAX).astype(FP8)


# ----------------------------------------------------------------------------
# Host-side layout preparation
# ----------------------------------------------------------------------------

def host_prep(ps, ln_gamma, ln_beta, W1, W2, W3, W_comp, species_idx, batch):
    ps = np.asarray(ps, dtype=np.float32)
    species_idx = np.asarray(species_idx).astype(np.int64)
    batch = np.asarray(batch).astype(np.int64)
    ln_gamma = np.asarray(ln_gamma, dtype=np.float32)
    ln_beta = np.asarray(ln_beta, dtype=np.float32)
    W1 = np.asarray(W1, dtype=np.float32)
    W2 = np.asarray(W2, dtype=np.float32)
    assert float(np.abs(ln_beta).max()) == 0.0, "kernel assumes ln_beta == 0"

    n_total = N_CORES * ATOMS_PER_CORE
    ps_pad = np.zeros((n_total, D_IN), dtype=np.float32)
    ps_pad[:N_ATOMS] = ps
    sp_pad = np.zeros(n_total, dtype=np.int64)
    sp_pad[:N_ATOMS] = species_idx
    sp_pad[N_ATOMS:] = np.arange(n_total - N_ATOMS) % N_SPECIES
    valid = np.zeros(n_total, dtype=bool)
    valid[:N_ATOMS] = True
    bt_pad = np.zeros(n_total, dtype=np.int64)
    bt_pad[:N_ATOMS] = batch

    counts = np.zeros((N_CORES, N_GROUPS, N_SPECIES), dtype=np.int64)
    for c in range(N_CORES):
        for g in range(N_GROUPS):
            lo = c * ATOMS_PER_CORE + g * GROUP_ATOMS
            counts[c, g] = np.bincount(sp_pad[lo:lo + GROUP_ATOMS],
                                       minlength=N_SPECIES)
    T_s = int(np.ceil(counts.max() / P))
    n_tiles = N_GROUPS * N_SPECIES * T_s

    group_min = np.zeros((N_CORES, N_GROUPS), dtype=np.int64)
    for c in range(N_CORES):
        for g in range(N_GROUPS):
            lo = c * ATOMS_PER_CORE + g * GROUP_ATOMS
            hi = lo + GROUP_ATOMS
            v = valid[lo:hi]
            if v.any():
                bts = bt_pad[lo:hi][v]
                group_min[c, g] = bts.min()
                assert int(bts.max() - bts.min() + 1) <= BINS
            else:
                group_min[c, g] = 0

    # fp8 atom features in DoubleRow layout [p, tile, c, j, atom]
    xt_all = np.zeros((N_CORES, P, n_tiles, 2, 2, P), dtype=FP8)
    # one-hot bins [atom, tile, bins]
    mh_all = np.zeros((N_CORES, P, n_tiles, BINS), dtype=FP8)

    for c in range(N_CORES):
        for g in range(N_GROUPS):
            lo = c * ATOMS_PER_CORE + g * GROUP_ATOMS
            hi = lo + GROUP_ATOMS
            sl_sp = sp_pad[lo:hi]
            order = np.argsort(sl_sp, kind="stable")
            gidx = np.arange(lo, hi)[order]
            gsp = sl_sp[order]
            for s in range(N_SPECIES):
                sel = gidx[gsp == s]
                cnt = len(sel)
                t0 = (g * N_SPECIES + s) * T_s
                blk = np.zeros((T_s * P, D_IN), dtype=np.float32)
                blk[:cnt] = ps_pad[sel]
                # feature f = 128*(2c+j) + p  ->  [t, a, c, j, p] -> [p, t, c, j, a]
                b5 = blk.reshape(T_s, P, 2, 2, P).transpose(4, 0, 2, 3, 1)
                xt_all[c, :, t0:t0 + T_s] = _q8(b5)
                vmask = valid[sel]
                rloc = (bt_pad[sel] - group_min[c, g]).astype(np.int64)
                mh = np.zeros((T_s * P, BINS), dtype=np.float32)
                rows = np.arange(cnt)[vmask]
                mh[rows, rloc[vmask]] = 1.0
                # [t, a, b] -> [a, t, b]
                mh_all[c, :, t0:t0 + T_s] = mh.reshape(T_s, P, BINS).transpose(1, 0, 2)

    # per-(structure, species) atom counts for the composition term (host side)
    counts_ss = np.zeros((N_STRUCT, N_SPECIES), dtype=np.float64)
    np.add.at(counts_ss, (batch, species_idx), 1.0)

    # Weights: fold gamma + exact mean-subtraction (rank-1 projection) into W1.
    w1r = np.zeros((N_SPECIES, 2, P, 2, HIDDEN), dtype=FP8)
    w2il = np.zeros((N_SPECIES, P, 2, HIDDEN), dtype=FP8)
    sW1 = np.zeros(N_SPECIES, dtype=np.float32)
    sW2 = np.zeros(N_SPECIES, dtype=np.float32)
    for s in range(N_SPECIES):
        W1g = ln_gamma[:, None] * W1[s]
        W1c = W1g - W1g.sum(0)[None, :] / D_IN
        sW1[s] = FP8_MAX / float(np.abs(W1c).max())
        # [f, h] with f = 128*(2c+j)+p -> [c, j, p, h] -> [c, p, j, h]
        w1r[s] = _q8((W1c * sW1[s]).reshape(2, 2, P, HIDDEN).transpose(0, 2, 1, 3))
        sW2[s] = FP8_MAX / float(np.abs(W2[s]).max())
        w2il[s] = _q8((W2[s] * sW2[s]).reshape(2, P, HIDDEN).transpose(1, 0, 2))

    # Pack every constant into one per-partition blob (single DMA):
    # [0:4096)    w1r   [s, c, j, h] fp8
    # [4096:6144) w2il  [s, j, h] fp8
    # [6144:6272) idn   row, 128 fp8
    # [6272:6320) scl   [s, k] f32: k=0: -0.5/sW1, k=1: 1.5/sW1, k=2: 1/sW2
    wb = np.zeros((P, 6320), dtype=np.uint8)
    wb[:, 0:4096] = w1r.transpose(2, 0, 1, 3, 4).reshape(P, 4096).view(np.uint8)
    wb[:, 4096:6144] = w2il.transpose(1, 0, 2, 3).reshape(P, 2048).view(np.uint8)
    wb[:, 6144:6272] = np.eye(P, dtype=FP8).view(np.uint8)
    scl = np.zeros((N_SPECIES, 3), dtype=np.float32)
    scl[:, 0] = -0.5 / sW1
    scl[:, 1] = 1.5 / sW1
    scl[:, 2] = 1.0 / sW2
    wb[:, 6272:6320] = np.broadcast_to(scl.reshape(-1).view(np.uint8), (P, 48))
    wb = wb.view(FP8)

    in_maps = []
    for c in range(N_CORES):
        in_maps.append({
            "xt": np.ascontiguousarray(xt_all[c]),
            "mh": np.ascontiguousarray(mh_all[c]),
            "wb": wb,
        })
    meta = dict(T_s=T_s, n_tiles=n_tiles, group_min=group_min,
                counts_ss=counts_ss)
    return in_maps, meta


# ----------------------------------------------------------------------------
# Device program
# ----------------------------------------------------------------------------

def build_program(T_s, bufs=None):
    import concourse.bacc as bacc
    import concourse.tile as tile
    from concourse import mybir

    n_tiles = N_GROUPS * N_SPECIES * T_s
    f32 = mybir.dt.float32
    bf16 = mybir.dt.bfloat16
    fp8 = mybir.dt.float8e4
    u16 = mybir.dt.uint16
    DR = mybir.MatmulPerfMode.DoubleRow
    SILU = mybir.ActivationFunctionType.Silu
    SQRT = mybir.ActivationFunctionType.Sqrt
    OP = mybir.AluOpType

    B = {"xt": 4, "mh": 4, "h1": 3, "h1t": 4, "h2": 3, "strip": 4, "csb": 2,
         "trash": 2, "p1": 3, "gps": 1, "p2": 2}
    B.update(bufs or {})

    nc = bacc.Bacc("TRN2", target_bir_lowering=False, debug=False,
                   num_devices=N_CORES)
    xt_d = nc.dram_tensor("xt", [P, n_tiles, 2, 2, P], fp8, kind="ExternalInput")
    mh_d = nc.dram_tensor("mh", [P, n_tiles, BINS], fp8, kind="ExternalInput")
    wb_d = nc.dram_tensor("wb", [P, 6320], fp8, kind="ExternalInput")
    out_d = nc.dram_tensor("c_out", [N_GROUPS * N_SPECIES, BINS, HIDDEN], f32,
                           kind="ExternalOutput")

    from contextlib import ExitStack
    with tile.TileContext(nc, trace_sim=False) as tc:
        with ExitStack() as ctx:
            singles = ctx.enter_context(tc.tile_pool(name="singles", bufs=1))
            xt_pool = ctx.enter_context(tc.tile_pool(name="xt", bufs=B["xt"]))
            mh_pool = ctx.enter_context(tc.tile_pool(name="mh", bufs=B["mh"]))
            h1_pool = ctx.enter_context(tc.tile_pool(name="h1", bufs=B["h1"]))
            h1t_pool = ctx.enter_context(tc.tile_pool(name="h1t", bufs=B["h1t"]))
            h2_pool = ctx.enter_context(tc.tile_pool(name="h2", bufs=B["h2"]))
            strip_pool = ctx.enter_context(tc.tile_pool(name="strip", bufs=B["strip"]))
            csb_pool = ctx.enter_context(tc.tile_pool(name="csb", bufs=B["csb"]))
            trash_pool = ctx.enter_context(tc.tile_pool(name="trash", bufs=B["trash"]))
            p1_pool = ctx.enter_context(tc.tile_pool(name="p1", bufs=B["p1"], space="PSUM"))
            gps_pool = ctx.enter_context(tc.tile_pool(name="gps", bufs=B["gps"], space="PSUM"))
            p2_pool = ctx.enter_context(tc.tile_pool(name="p2", bufs=B["p2"], space="PSUM"))
            shared_pool = ctx.enter_context(tc.tile_pool(name="shared", bufs=1, space="PSUM"))
            cps_pool = ctx.enter_context(tc.tile_pool(name="cps", bufs=1, space="PSUM"))

            # load all constants in one DMA (ACT queue, parallel with xt on SP)
            WB = singles.tile([P, 6320], fp8)
            nc.scalar.dma_start(WB[:], wb_d.ap())

            def w1_ap(s, c):     # [p, j, h] fp8, k-pair c of species s
                o = (s * 2 + c) * 512
                return WB[:, o:o + 512].rearrange("p (j h) -> p j h", j=2)

            def w2_ap(s):        # [p, j, h] fp8
                o = 4096 + s * 512
                return WB[:, o:o + 512].rearrange("p (j h) -> p j h", j=2)

            IDN = WB[:, 6144:6272]                   # [P, 128] fp8 identity

            def scl_ap(s, k):    # [P, 1] f32 per-partition constant
                o = 6272 + (s * 3 + k) * 4
                return WB[:, o:o + 4].bitcast(f32)

            blocks = [(g, s) for g in range(N_GROUPS) for s in range(N_SPECIES)]
            n_quads = (T_s + 3) // 4
            # ascending quad sizes (e.g. 13 -> [2,3,4,4]): few odd singles,
            # and a big B stage right before each block boundary so the next
            # block's stats chain is hidden
            qsz = B.get("qsz") or [min(4, T_s - 4 * i) for i in range(n_quads)]
            n_quads = len(qsz)
            qlo = [sum(qsz[:i]) for i in range(n_quads)]
            # transpose slots get a bank of their own: a start=True matmul
            # group resets same-bank accumulation state on HW, so the
            # long-lived CPS segment accumulator must not share a bank with
            # the transpose groups.
            SH = shared_pool.tile([P, 512], u16)
            CPS_T = cps_pool.tile([BINS, HIDDEN], f32)
            CPS = CPS_T[:]
            quads = [(bi, qd) for bi in range(len(blocks)) for qd in range(n_quads)]
            states = {}
            qstate = {}
            tcnt = [0]
            DEPTH = B.get("depth", 3)

            def dma_in(bi):
                if bi >= len(blocks):
                    return
                t0 = bi * T_s
                XTB = xt_pool.tile([P, T_s, 2, 2, P], fp8)
                if bi == 0:
                    # split so the first quad's compute starts early
                    for qd in range(n_quads):
                        lo, hi = qlo[qd], qlo[qd] + qsz[qd]
                        nc.sync.dma_start(XTB[:, lo:hi],
                                          xt_d.ap()[:, t0 + lo:t0 + hi])
                else:
                    nc.sync.dma_start(XTB[:], xt_d.ap()[:, t0:t0 + T_s])
                MH = mh_pool.tile([P, T_s, BINS], fp8)
                nc.sync.dma_start(MH[:], mh_d.ap()[:, t0:t0 + T_s])
                states[bi] = dict(XTB=XTB, MH=MH)

            def stage_a(qi):
                bi, qd = quads[qi]
                g, s = blocks[bi]
                if qd == 0:
                    dma_in(bi + DEPTH)
                XTB = states[bi]["XTB"]
                ts = list(range(qlo[qd], qlo[qd] + qsz[qd]))
                nt = len(ts)
                SUMQ = strip_pool.tile([P, 8], f32, tag="sumq")
                P1s = []
                for pi in range((nt + 1) // 2):
                    pts = ts[2 * pi:2 * pi + 2]
                    P1 = p1_pool.tile([P, 2, HIDDEN], f32)
                    P1s.append(P1)
                    GPS = gps_pool.tile([P, 2, P], f32)
                    for i2, t in enumerate(pts):
                        i = 2 * pi + i2
                        for c in range(2):
                            nc.tensor.matmul(P1[:, i2, :], XTB[:, t, c],
                                             w1_ap(s, c), start=(c == 0),
                                             stop=(c == 1), perf_mode=DR)
                        for c in range(2):
                            nc.tensor.matmul(GPS[:, i2, :], XTB[:, t, c],
                                             XTB[:, t, c], start=(c == 0),
                                             stop=(c == 1), perf_mode=DR)
                        TR = trash_pool.tile([P, P], bf16)
                        nc.vector.scalar_tensor_tensor(
                            TR[:], GPS[:, i2, :], 1.0, IDN, OP.bypass,
                            OP.mult, accum_out=SUMQ[:, i:i + 1])
                # rs = rsqrt(q/512 + eps)/sW1: quadratic seed (Taylor at 1)
                # + one fused Newton step; last step folds 1/sW1 via
                # per-partition constant scalars.
                VQ = strip_pool.tile([P, 12], f32, tag="vq")
                V_, Y_, U_ = VQ[:, 0:nt], VQ[:, 4:4 + nt], VQ[:, 8:8 + nt]
                VS = SUMQ[:, 4:4 + nt]
                nc.vector.tensor_scalar(V_, SUMQ[:, 0:nt], 1.0 / D_IN, LN_EPS,
                                        OP.mult, OP.add)
                nc.vector.tensor_scalar(Y_, V_, 0.375, -1.25, OP.mult, OP.add)
                nc.vector.scalar_tensor_tensor(Y_, Y_, 1.0, V_, OP.bypass, OP.mult)
                nc.vector.tensor_scalar(Y_, Y_, 1.875, None, OP.add)
                nc.vector.scalar_tensor_tensor(U_, Y_, 1.0, Y_, OP.bypass, OP.mult)
                nc.vector.scalar_tensor_tensor(U_, U_, scl_ap(s, 0), V_,
                                               OP.mult, OP.mult)
                nc.vector.scalar_tensor_tensor(VS, U_, scl_ap(s, 1), Y_,
                                               OP.add, OP.mult)
                qstate[qi] = dict(P1s=P1s, VS=VS)

            pend_act = []   # pairs awaiting silu2
            pend_pe = []    # pairs awaiting segment matmul

            def flush_act():
                bi2, s2, P2, H2, pts2 = pend_act.pop(0)
                npt = len(pts2)
                nc.scalar.activation(H2[:, 0:npt, :], P2[:, 0:npt, :], SILU,
                                     scale=scl_ap(s2, 2))
                pend_pe.append((bi2, H2, pts2, 0))

            def flush_pe():
                bi2, H2, pts2, h0 = pend_pe.pop(0)
                MH2 = states[bi2]["MH"]
                npt = len(pts2)
                t = pts2[0]
                if npt == 2:
                    nc.tensor.matmul(CPS, MH2[:, t:t + 2, :],
                                     H2[:, h0:h0 + 2, :],
                                     start=(t == 0), stop=(t + 1 == T_s - 1),
                                     perf_mode=DR)
                else:
                    nc.tensor.matmul(CPS, MH2[:, t, :], H2[:, h0, :],
                                     start=(t == 0), stop=True)

            def stage_b(qi):
                bi, qd = quads[qi]
                g, s = blocks[bi]
                ts = list(range(qlo[qd], qlo[qd] + qsz[qd]))
                nt = len(ts)
                P1s, VS = qstate[qi]["P1s"], qstate[qi]["VS"]
                for pi in range((nt + 1) // 2):
                    pts = ts[2 * pi:2 * pi + 2]
                    npt = len(pts)
                    P1 = P1s[pi]
                    H1 = h1_pool.tile([P, 2, HIDDEN], fp8)
                    P2 = p2_pool.tile([P, 2, HIDDEN], f32)
                    H2 = h2_pool.tile([P, 2, HIDDEN], fp8)
                    if len(pend_act) >= 2:
                        flush_act()
                    tcnt[0] += 1
                    for i2 in range(npt):
                        i = 2 * pi + i2
                        nc.scalar.activation(H1[:, i2, :], P1[:, i2, :], SILU,
                                             scale=VS[:, i:i + 1])
                    for i2 in range(npt):
                        # fp8 transpose writes element-step-2 (u16 lanes);
                        # chunk k gets its own 128-lane region
                        for k in range(2):
                            o = 256 * i2 + 128 * k
                            TPK = SH[:, o:o + 128].bitcast(fp8).rearrange(
                                "p (a b) -> p b a", b=2)[:, 0, :]
                            nc.tensor.transpose(
                                TPK, H1[:, i2, 128 * k:128 * (k + 1)], IDN)
                    H1T = h1t_pool.tile([P, 4, P], u16)
                    nc.vector.tensor_copy(H1T[:, 0:2 * npt, :],
                                          SH[:, 0:256 * npt])
                    for i2 in range(npt):
                        nc.tensor.matmul(
                            P2[:, i2, :],
                            H1T[:, 2 * i2:2 * i2 + 2, :].bitcast(fp8).rearrange(
                                "p j (a b) -> p j b a", b=2)[:, :, 0, :],
                            w2_ap(s), start=True, stop=True, perf_mode=DR)
                    while pend_pe:
                        flush_pe()
                    pend_act.append((bi, s, P2, H2, pts))
                qstate.pop(qi)
                if qd == n_quads - 1:
                    while pend_act:
                        flush_act()
                    while pend_pe:
                        flush_pe()
                    CSB = csb_pool.tile([BINS, HIDDEN], f32)
                    nc.vector.tensor_copy(CSB[:], CPS)
                    nc.sync.dma_start(out_d.ap()[bi], CSB[:])

            for a in range(min(DEPTH, len(blocks))):
                dma_in(a)
            stage_a(0)
            for qi in range(len(quads)):
                if qi + 1 < len(quads):
                    stage_a(qi + 1)
                stage_b(qi)
            while pend_act:
                flush_act()
            while pend_pe:
                flush_pe()

    nc.compile()
    return nc


# ----------------------------------------------------------------------------
# Aggregation
# ----------------------------------------------------------------------------

def aggregate(results, meta, W3, W_comp):
    W3 = np.asarray(W3, dtype=np.float64)
    W_comp = np.asarray(W_comp, dtype=np.float64)
    group_min = meta["group_min"]
    E = np.zeros(N_STRUCT, dtype=np.float64)
    for c in range(N_CORES):
        cout = np.asarray(results[c]["c_out"], dtype=np.float64)  # [16, bins, 256]
        for g in range(N_GROUPS):
            b0 = int(group_min[c, g])
            nb = min(BINS, N_STRUCT - b0)
            for s in range(N_SPECIES):
                blk = cout[g * N_SPECIES + s]
                E[b0:b0 + nb] += blk[:nb] @ W3[s][:, 0]
    energies = (E / AVG_N_ATOMS)[:, None] * E_SCALE \
        + meta["counts_ss"] @ W_comp.T
    return energies.astype(np.float32)


# ----------------------------------------------------------------------------
# Entry point
# ----------------------------------------------------------------------------

_PROGRAM_CACHE = {}


def kernel(ps, ln_gamma, ln_beta, W1, W2, W3, W_comp, species_idx, batch):
    from concourse import bass_utils

    in_maps, meta = host_prep(ps, ln_gamma, ln_beta, W1, W2, W3, W_comp,
                              species_idx, batch)
    key = meta["T_s"]
    if key not in _PROGRAM_CACHE:
        _PROGRAM_CACHE[key] = build_program(meta["T_s"])
    nc = _PROGRAM_CACHE[key]
    res = bass_utils.run_bass_kernel_spmd(nc, in_maps,
                                          core_ids=list(range(N_CORES)))
    return aggregate(res.results, meta, W3, W_comp)



# revision 28
# speedup vs baseline: 1.7326x; 1.7326x over previous
"""Trainium2 Bass kernel for BPPS model (LayerNorm -> per-species MLP -> segment sum).

Self-contained: hardcodes shapes from the problem spec.
  ps [200000, 512] f32, species_idx [200000] int, batch [200000] int (sorted),
  ln_gamma/ln_beta [512], W1 [4,512,256], W2 [4,256,256], W3 [4,256,1], W_comp [1,4].
Output: energies [2000, 1] f32.

Strategy: data-parallel over atoms on 8 NeuronCores (25000 atoms/core).
Host does layout + per-atom LayerNorm scalars; device does all GEMMs and SiLUs.

Device pipeline (per 4-tile "quad" of 512 atoms, [hidden_p, atoms] layout):
  L1: 16 fp8 DoubleRow matmuls (W1 stationary, X moving) -> P1 [128, 2, 512] f32
      (one 2-bank PSUM tile); one big SiLU on ACT ([128,1024] PSUM -> fp8 SBUF);
  L2: 8 fp8 DR matmuls (W2 stationary, H1 moving) -> P2; SiLU -> H2 bf16;
  W3: 8 near-free bf16 matmuls with tiny [128,1] outputs -> per-atom energies;
  energies accumulate in SBUF, one DMA out; segment-sum on host.
No transposes, no one-hot segment matmul, no on-device LayerNorm stats.
"""

import sys

sys.path.insert(0, "/opt/trn_rl_repo")

import numpy as np
import ml_dtypes

BF16 = ml_dtypes.bfloat16
FP8 = ml_dtypes.float8_e4m3

# Problem constants
N_ATOMS = 200000
D_IN = 512
HIDDEN = 256
N_SPECIES = 4
N_STRUCT = 2000
AVG_N_ATOMS = 60.0
E_SCALE = 1.0
LN_EPS = 1e-5

N_CORES = 8
APC = N_ATOMS // N_CORES        # 25000 atoms per core
P = 128
FP8_MAX = 224.0                 # headroom under trn e4m3 max (240)


def _q8(x):
    return np.clip(x, -FP8_MAX, FP8_MAX).astype(FP8)


# ----------------------------------------------------------------------------
# Host-side layout preparation
# ----------------------------------------------------------------------------

def host_prep(ps, ln_gamma, ln_beta, W1, W2, W3, W_comp, species_idx, batch):
    ps = np.asarray(ps, dtype=np.float32)
    species_idx = np.asarray(species_idx).astype(np.int64)
    batch = np.asarray(batch).astype(np.int64)
    ln_gamma = np.asarray(ln_gamma, dtype=np.float32)
    ln_beta = np.asarray(ln_beta, dtype=np.float32)
    W1 = np.asarray(W1, dtype=np.float32)
    W2 = np.asarray(W2, dtype=np.float32)
    W3f = np.asarray(W3, dtype=np.float32)

    # exact LayerNorm on host (host already touches every element to quantize)
    mu = ps.mean(axis=1, keepdims=True)
    var = ps.var(axis=1, keepdims=True)
    x = (ps - mu) * (1.0 / np.sqrt(var + LN_EPS)) * ln_gamma + ln_beta

    sX = FP8_MAX / float(np.abs(x).max())
    sW1 = FP8_MAX / float(np.abs(W1).max())
    sW2 = FP8_MAX / float(np.abs(W2).max())
    scale1 = 1.0 / (sX * sW1)
    scale2 = 1.0 / sW2

    # Global species sort, atoms dealt per-species evenly across cores: any
    # atom may go to any core (segment-sum happens on host), so per-core
    # per-species counts are equal +-1 and the shared SPMD tile map is minimal.
    order_g = np.argsort(species_idx, kind="stable")
    n_sp = np.bincount(species_idx, minlength=N_SPECIES)
    pool_off = np.cumsum([0] + list(n_sp))
    cnt_cs = np.zeros((N_CORES, N_SPECIES), dtype=np.int64)
    for s in range(N_SPECIES):
        q, r = divmod(int(n_sp[s]), N_CORES)
        cnt_cs[:, s] = q
        cnt_cs[:r, s] += 1
    T_sp = [int(np.ceil(cnt_cs[:, s].max() / P)) for s in range(N_SPECIES)]
    T = sum(T_sp)
    species_map = []
    for s in range(N_SPECIES):
        species_map += [s] * T_sp[s]
    offs = np.cumsum([0] + T_sp)     # tile offset of each species block

    xq = _q8(x * sX)
    xt_all = np.zeros((N_CORES, P, T, 2, 2, P), dtype=FP8)
    perms = np.full((N_CORES, T * P), -1, dtype=np.int64)
    for c in range(N_CORES):
        xpad = np.zeros((T * P, D_IN), dtype=FP8)
        for s in range(N_SPECIES):
            cnt = int(cnt_cs[c, s])
            p0 = int(pool_off[s]) + int(cnt_cs[:c, s].sum())
            sel = order_g[p0:p0 + cnt]
            s0 = int(offs[s]) * P
            xpad[s0:s0 + cnt] = xq[sel]
            perms[c, s0:s0 + cnt] = sel
        # feature f = 256*c + 128*j + p : [t, a, c, j, p] -> [p, t, c, j, a]
        xt_all[c] = xpad.reshape(T, P, 2, 2, P).transpose(4, 0, 2, 3, 1)

    # per-(structure, species) atom counts for the composition term
    counts_ss = np.zeros((N_STRUCT, N_SPECIES), dtype=np.float64)
    np.add.at(counts_ss, (batch, species_idx), 1.0)

    # Weights, [hidden_p, *] stationary layouts.
    # w1r[s, c, m1] = [p, j, m] fp8 from W1[f= 256c+128j+p, h= 128*m1+m]
    w1r = _q8((W1 * sW1).reshape(N_SPECIES, 2, 2, P, 2, P)
              .transpose(0, 1, 4, 3, 2, 5))          # [s, c, m1, p, j, m]
    # w2r[s, m1] = [p, jj, m] from W2[h1= 128*jj+p, h2= 128*m1+m]
    w2r = _q8((W2 * sW2).reshape(N_SPECIES, 2, P, 2, P)
              .transpose(0, 3, 2, 1, 4))             # [s, m1, p, jj, m]
    # w3b[s, m1] = [p] bf16 from W3[h2 = 128*m1+p]
    w3b = W3f[:, :, 0].reshape(N_SPECIES, 2, P).astype(BF16)  # [s, m1, p]

    # one per-partition constant blob (single DMA):
    # [0:4096)    w1r  [s, c, m1][j, m] fp8
    # [4096:6144) w2r  [s, m1][jj, m] fp8
    # [6144:6160) w3b  [s, m1] bf16
    wb = np.zeros((P, 6160), dtype=np.uint8)
    wb[:, 0:4096] = w1r.transpose(3, 0, 1, 2, 4, 5).reshape(P, 4096).view(np.uint8)
    wb[:, 4096:6144] = w2r.transpose(2, 0, 1, 3, 4).reshape(P, 2048).view(np.uint8)
    wb[:, 6144:6160] = np.ascontiguousarray(
        w3b.transpose(2, 0, 1)).reshape(P, 8).view(np.uint8)
    wb = wb.view(FP8)

    in_maps = []
    for c in range(N_CORES):
        in_maps.append({
            "xt": np.ascontiguousarray(xt_all[c]),
            "wb": wb,
        })
    meta = dict(T=T, species_map=tuple(species_map), perms=perms,
                counts_ss=counts_ss, scale1=scale1, scale2=scale2)
    return in_maps, meta


# ----------------------------------------------------------------------------
# Device program
# ----------------------------------------------------------------------------

def build_program(T, species_map, scale1, scale2, bufs=None):
    import concourse.bacc as bacc
    import concourse.tile as tile
    from concourse import mybir

    B = {"depth": 3, "gs": 8, "h1": 3, "h2": 3}
    B.update(bufs or {})

    f32 = mybir.dt.float32
    bf16 = mybir.dt.bfloat16
    fp8 = mybir.dt.float8e4
    DR = mybir.MatmulPerfMode.DoubleRow
    SILU = mybir.ActivationFunctionType.Silu

    GS = B["gs"]                      # tiles per group (8 => 4-bank psum tiles)
    G = (T + GS - 1) // GS
    DEPTH = B["depth"]
    EO = GS * P - 8                   # e scratch corner in the p2 slot (bank GS//2-1)

    def g_tiles(g):
        return min(GS, T - GS * g)

    nc = bacc.Bacc("TRN2", target_bir_lowering=False, debug=False,
                   num_devices=N_CORES)
    xt_d = nc.dram_tensor("xt", [P, T, 2, 2, P], fp8, kind="ExternalInput")
    wb_d = nc.dram_tensor("wb", [P, 6160], fp8, kind="ExternalInput")
    out_d = nc.dram_tensor("e_out", [P, T], f32, kind="ExternalOutput")

    from contextlib import ExitStack
    with tile.TileContext(nc, trace_sim=False) as tc:
        with ExitStack() as ctx:
            singles = ctx.enter_context(tc.tile_pool(name="singles", bufs=1))
            xt_pool = ctx.enter_context(tc.tile_pool(name="xt", bufs=DEPTH + 1))
            h1_pool = ctx.enter_context(tc.tile_pool(name="h1", bufs=B["h1"]))
            h2_pool = ctx.enter_context(tc.tile_pool(name="h2", bufs=B["h2"]))
            psum_s = ctx.enter_context(tc.tile_pool(name="ps", bufs=1, space="PSUM"))
            # static psum tiles; subtile dependency tracking orders reuse
            P1S = psum_s.tile([P, 2, GS * P], f32, tag="p1s")
            P2S = psum_s.tile([P, 2, GS * P], f32, tag="p2s")

            # dummy first activation: hoists the Silu table load to t=0 on the
            # otherwise-idle ACT queue (it would otherwise land right before
            # the first real silu, on the critical path)
            DUM = singles.tile([P, 1], f32)
            nc.scalar.activation(DUM[:], DUM[:], SILU)

            WB = singles.tile([P, 6160], fp8)
            # species-0 W1 chunk first on the SP queue, ahead of the xt
            # prefetches, so L1(0) starts as early as possible
            nc.sync.dma_start(WB[:, 0:1024], wb_d.ap()[:, 0:1024])
            E = singles.tile([P, T], f32)

            def w1_ap(s, c, m1):     # [p, j, m] fp8 stationary chunk
                o = ((s * 2 + c) * 2 + m1) * 256
                return WB[:, o:o + 256].rearrange("p (j m) -> p j m", j=2)

            def w2_ap(s, m1):        # [p, jj, m] fp8
                o = 4096 + (s * 2 + m1) * 256
                return WB[:, o:o + 256].rearrange("p (jj m) -> p jj m", jj=2)

            def w3_ap(s, m1):        # [p, 1] bf16
                o = 6144 + (s * 2 + m1) * 2
                return WB[:, o:o + 2].bitcast(bf16)

            xts = {}

            def dma_in(g):
                if g >= G:
                    return
                tl = g_tiles(g)
                XT = xt_pool.tile([P, GS, 2, 2, P], fp8)
                if g == 0:
                    # split so the first tiles' compute starts early
                    for i, hi in ((0, 2), (2, tl)):
                        if hi > i:
                            nc.sync.dma_start(XT[:, i:hi],
                                              xt_d.ap()[:, GS * g + i:GS * g + hi])
                else:
                    nc.sync.dma_start(XT[:, 0:tl],
                                      xt_d.ap()[:, GS * g:GS * g + tl])
                xts[g] = XT

            for a in range(DEPTH):
                dma_in(a)
            nc.sync.dma_start(WB[:, 1024:4096], wb_d.ap()[:, 1024:4096])
            nc.sync.dma_start(WB[:, 4096:6160], wb_d.ap()[:, 4096:6160])

            def w3_group(g2, dst):
                # per-atom energies of group g2 into a psum corner + E copy
                H2 = h2s.pop(g2)
                tl2 = g_tiles(g2)
                for i in range(tl2):
                    s = species_map[GS * g2 + i]
                    for m1 in range(2):
                        nc.tensor.matmul(
                            dst[:, 1, EO + i:EO + i + 1],
                            H2[:, m1, P * i:P * (i + 1)], w3_ap(s, m1),
                            start=(m1 == 0), stop=(m1 == 1))
                lo = GS * g2
                nc.vector.tensor_copy(E[:, lo:lo + tl2], dst[:, 1, EO:EO + tl2])
                return lo, tl2

            h1s, h2s = {}, {}
            for g in range(G + 2):
                if g < G:
                    dma_in(g + DEPTH)
                    XT = xts.pop(g)
                    tl = g_tiles(g)
                    for i in range(tl):
                        s = species_map[GS * g + i]
                        for m1 in range(2):
                            for c in range(2):
                                nc.tensor.matmul(
                                    P1S[:, m1, P * i:P * (i + 1)],
                                    w1_ap(s, c, m1), XT[:, i, c],
                                    start=(c == 0), stop=(c == 1),
                                    perf_mode=DR)
                if 2 <= g < G - 1:
                    # W3 of the group silu2'd one ACT slot ago; e lands in a
                    # dead corner of the p2 slot (idle between silu2(g-2)'s
                    # read and L2(g-1)'s write). Its dependency (silu2(g-2))
                    # is long done, so this never delays the ACT queue.
                    w3_group(g - 2, P2S)
                if 1 <= g <= G:
                    tl = g_tiles(g - 1)
                    H1p = h1s.pop(g - 1)
                    H2 = h2_pool.tile([P, 2, GS * P], bf16)
                    for i in range(tl):
                        s = species_map[GS * (g - 1) + i]
                        for m1 in range(2):
                            nc.tensor.matmul(
                                P2S[:, m1, P * i:P * (i + 1)],
                                w2_ap(s, m1), H1p[:, :, P * i:P * (i + 1)],
                                start=True, stop=True, perf_mode=DR)
                    nc.scalar.activation(H2[:, :, 0:P * tl], P2S[:, :, 0:P * tl],
                                         SILU, scale=scale2)
                    h2s[g - 1] = H2
                if g < G:
                    tl = g_tiles(g)
                    H1 = h1_pool.tile([P, 2, GS * P], fp8)
                    if g == 0 and tl > 2:
                        # split: fill the ACT pipe before all of group 0 lands
                        h = 2 * P
                        nc.scalar.activation(H1[:, :, 0:h], P1S[:, :, 0:h],
                                             SILU, scale=scale1)
                        nc.scalar.activation(H1[:, :, h:P * tl],
                                             P1S[:, :, h:P * tl],
                                             SILU, scale=scale1)
                    else:
                        nc.scalar.activation(H1[:, :, 0:P * tl],
                                             P1S[:, :, 0:P * tl],
                                             SILU, scale=scale1)
                    h1s[g] = H1
                if g >= G - 1 and g >= 2:
                    # tail groups: s1(G-1) has issued, so P1S's corner is (or
                    # will be) dead; W3 goes there after L2/silu2 above,
                    # keeping the final ACT slots back-to-back.
                    lo, tl2 = w3_group(g - 2, P1S)
                    if g == G - 1:
                        # everything through group G-3 is final
                        nc.sync.dma_start(out_d.ap()[:, 0:lo + tl2],
                                          E[:, 0:lo + tl2])
                    else:
                        nc.sync.dma_start(out_d.ap()[:, lo:lo + tl2],
                                          E[:, lo:lo + tl2])

    nc.compile()
    return nc


# ----------------------------------------------------------------------------
# Aggregation
# ----------------------------------------------------------------------------

def aggregate(results, meta, batch, species_idx, W_comp):
    W_comp = np.asarray(W_comp, dtype=np.float64)
    batch = np.asarray(batch).astype(np.int64)
    perms = meta["perms"]
    E = np.zeros(N_STRUCT, dtype=np.float64)
    for c in range(N_CORES):
        e = np.asarray(results[c]["e_out"], dtype=np.float64)  # [128, T]
        e_slots = e.T.reshape(-1)                 # slot = t*128 + lane
        pm = perms[c]
        valid = pm >= 0
        E += np.bincount(batch[pm[valid]], weights=e_slots[valid],
                         minlength=N_STRUCT)
    energies = (E / AVG_N_ATOMS)[:, None] * E_SCALE \
        + meta["counts_ss"] @ W_comp.T
    return energies.astype(np.float32)


# ----------------------------------------------------------------------------
# Entry point
# ----------------------------------------------------------------------------

_PROGRAM_CACHE = {}


def kernel(ps, ln_gamma, ln_beta, W1, W2, W3, W_comp, species_idx, batch):
    from concourse import bass_utils

    in_maps, meta = host_prep(ps, ln_gamma, ln_beta, W1, W2, W3, W_comp,
                              species_idx, batch)
    key = (meta["T"], meta["species_map"], meta["scale1"], meta["scale2"])
    if key not in _PROGRAM_CACHE:
        _PROGRAM_CACHE[key] = build_program(meta["T"], meta["species_map"],
                                            meta["scale1"], meta["scale2"])
    nc = _PROGRAM_CACHE[key]
    res = bass_utils.run_bass_kernel_spmd(nc, in_maps,
                                          core_ids=list(range(N_CORES)))
    return aggregate(res.results, meta, batch, species_idx, W_comp)


# revision 43
# speedup vs baseline: 1.7450x; 1.0072x over previous
"""Trainium2 Bass kernel for BPPS model (LayerNorm -> per-species MLP -> segment sum).

Self-contained: hardcodes shapes from the problem spec.
  ps [200000, 512] f32, species_idx [200000] int, batch [200000] int (sorted),
  ln_gamma/ln_beta [512], W1 [4,512,256], W2 [4,256,256], W3 [4,256,1], W_comp [1,4].
Output: energies [2000, 1] f32.

Strategy: data-parallel over atoms on 8 NeuronCores (25000 atoms/core).
Host does layout + per-atom LayerNorm scalars; device does all GEMMs and SiLUs.

Device pipeline (per 4-tile "quad" of 512 atoms, [hidden_p, atoms] layout):
  L1: 16 fp8 DoubleRow matmuls (W1 stationary, X moving) -> P1 [128, 2, 512] f32
      (one 2-bank PSUM tile); one big SiLU on ACT ([128,1024] PSUM -> fp8 SBUF);
  L2: 8 fp8 DR matmuls (W2 stationary, H1 moving) -> P2; SiLU -> H2 bf16;
  W3: 8 near-free bf16 matmuls with tiny [128,1] outputs -> per-atom energies;
  energies accumulate in SBUF, one DMA out; segment-sum on host.
No transposes, no one-hot segment matmul, no on-device LayerNorm stats.
"""

import sys

sys.path.insert(0, "/opt/trn_rl_repo")

import numpy as np
import ml_dtypes

BF16 = ml_dtypes.bfloat16
FP8 = ml_dtypes.float8_e4m3

# Problem constants
N_ATOMS = 200000
D_IN = 512
HIDDEN = 256
N_SPECIES = 4
N_STRUCT = 2000
AVG_N_ATOMS = 60.0
E_SCALE = 1.0
LN_EPS = 1e-5

N_CORES = 8
APC = N_ATOMS // N_CORES        # 25000 atoms per core
P = 128
FP8_MAX = 224.0                 # headroom under trn e4m3 max (240)


def _q8(x):
    return np.clip(x, -FP8_MAX, FP8_MAX).astype(FP8)


# ----------------------------------------------------------------------------
# Host-side layout preparation
# ----------------------------------------------------------------------------

def host_prep(ps, ln_gamma, ln_beta, W1, W2, W3, W_comp, species_idx, batch):
    ps = np.asarray(ps, dtype=np.float32)
    species_idx = np.asarray(species_idx).astype(np.int64)
    batch = np.asarray(batch).astype(np.int64)
    ln_gamma = np.asarray(ln_gamma, dtype=np.float32)
    ln_beta = np.asarray(ln_beta, dtype=np.float32)
    W1 = np.asarray(W1, dtype=np.float32)
    W2 = np.asarray(W2, dtype=np.float32)
    W3f = np.asarray(W3, dtype=np.float32)

    # exact LayerNorm on host (host already touches every element to quantize)
    mu = ps.mean(axis=1, keepdims=True)
    var = ps.var(axis=1, keepdims=True)
    x = (ps - mu) * (1.0 / np.sqrt(var + LN_EPS)) * ln_gamma + ln_beta

    sX = FP8_MAX / float(np.abs(x).max())
    sW1 = FP8_MAX / float(np.abs(W1).max())
    sW2 = FP8_MAX / float(np.abs(W2).max())
    scale1 = 1.0 / (sX * sW1)
    scale2 = 1.0 / sW2

    # Global species sort, atoms dealt per-species evenly across cores: any
    # atom may go to any core (segment-sum happens on host), so per-core
    # per-species counts are equal +-1 and the shared SPMD tile map is minimal.
    order_g = np.argsort(species_idx, kind="stable")
    n_sp = np.bincount(species_idx, minlength=N_SPECIES)
    pool_off = np.cumsum([0] + list(n_sp))
    cnt_cs = np.zeros((N_CORES, N_SPECIES), dtype=np.int64)
    for s in range(N_SPECIES):
        q, r = divmod(int(n_sp[s]), N_CORES)
        cnt_cs[:, s] = q
        cnt_cs[:r, s] += 1
    T_sp = [int(np.ceil(cnt_cs[:, s].max() / P)) for s in range(N_SPECIES)]
    T = sum(T_sp)
    species_map = []
    for s in range(N_SPECIES):
        species_map += [s] * T_sp[s]
    offs = np.cumsum([0] + T_sp)     # tile offset of each species block

    xq = _q8(x * sX)
    xt_all = np.zeros((N_CORES, P, T, 2, 2, P), dtype=FP8)
    perms = np.full((N_CORES, T * P), -1, dtype=np.int64)
    for c in range(N_CORES):
        xpad = np.zeros((T * P, D_IN), dtype=FP8)
        for s in range(N_SPECIES):
            cnt = int(cnt_cs[c, s])
            p0 = int(pool_off[s]) + int(cnt_cs[:c, s].sum())
            sel = order_g[p0:p0 + cnt]
            s0 = int(offs[s]) * P
            xpad[s0:s0 + cnt] = xq[sel]
            perms[c, s0:s0 + cnt] = sel
        # feature f = 256*c + 128*j + p : [t, a, c, j, p] -> [p, t, c, j, a]
        xt_all[c] = xpad.reshape(T, P, 2, 2, P).transpose(4, 0, 2, 3, 1)

    # per-(structure, species) atom counts for the composition term
    counts_ss = np.zeros((N_STRUCT, N_SPECIES), dtype=np.float64)
    np.add.at(counts_ss, (batch, species_idx), 1.0)

    # Weights, [hidden_p, *] stationary layouts.
    # w1r[s, c, m1] = [p, j, m] fp8 from W1[f= 256c+128j+p, h= 128*m1+m]
    w1r = _q8((W1 * sW1).reshape(N_SPECIES, 2, 2, P, 2, P)
              .transpose(0, 1, 4, 3, 2, 5))          # [s, c, m1, p, j, m]
    # w2r[s, m1] = [p, jj, m] from W2[h1= 128*jj+p, h2= 128*m1+m]
    w2r = _q8((W2 * sW2).reshape(N_SPECIES, 2, P, 2, P)
              .transpose(0, 3, 2, 1, 4))             # [s, m1, p, jj, m]
    # w3b[s, m1] = [p] bf16 from W3[h2 = 128*m1+p]
    w3b = W3f[:, :, 0].reshape(N_SPECIES, 2, P).astype(BF16)  # [s, m1, p]

    # one per-partition constant blob (single DMA):
    # [0:4096)    w1r  [s, c, m1][j, m] fp8
    # [4096:6144) w2r  [s, m1][jj, m] fp8
    # [6144:6160) w3b  [s, m1] bf16
    wb = np.zeros((P, 6160), dtype=np.uint8)
    wb[:, 0:4096] = w1r.transpose(3, 0, 1, 2, 4, 5).reshape(P, 4096).view(np.uint8)
    wb[:, 4096:6144] = w2r.transpose(2, 0, 1, 3, 4).reshape(P, 2048).view(np.uint8)
    wb[:, 6144:6160] = np.ascontiguousarray(
        w3b.transpose(2, 0, 1)).reshape(P, 8).view(np.uint8)
    wb = wb.view(FP8)

    in_maps = []
    for c in range(N_CORES):
        in_maps.append({
            "xt": np.ascontiguousarray(xt_all[c]),
            "wb": wb,
        })
    meta = dict(T=T, species_map=tuple(species_map), perms=perms,
                counts_ss=counts_ss, scale1=scale1, scale2=scale2)
    return in_maps, meta


# ----------------------------------------------------------------------------
# Device program
# ----------------------------------------------------------------------------

def build_program(T, species_map, scale1, scale2, bufs=None):
    import concourse.bacc as bacc
    import concourse.tile as tile
    from concourse import mybir

    B = {"depth": 3, "gs": 8, "h1": 3, "h2": 3}
    B.update(bufs or {})

    f32 = mybir.dt.float32
    bf16 = mybir.dt.bfloat16
    fp8 = mybir.dt.float8e4
    DR = mybir.MatmulPerfMode.DoubleRow
    SILU = mybir.ActivationFunctionType.Silu

    GS = B["gs"]                      # max tiles per group (8 => 4-bank psum tiles)
    DEPTH = B["depth"]
    EO = GS * P - 8                   # e scratch corner in a psum slot's last bank

    bounds = []
    t0 = 0
    while t0 < T:
        t1 = min(t0 + GS, T)
        bounds.append((t0, t1))
        t0 = t1
    G = len(bounds)

    nc = bacc.Bacc("TRN2", target_bir_lowering=False, debug=False,
                   num_devices=N_CORES)
    xt_d = nc.dram_tensor("xt", [P, T, 2, 2, P], fp8, kind="ExternalInput")
    wb_d = nc.dram_tensor("wb", [P, 6160], fp8, kind="ExternalInput")
    out_d = nc.dram_tensor("e_out", [P, T], f32, kind="ExternalOutput")

    from contextlib import ExitStack
    with tile.TileContext(nc, trace_sim=False) as tc:
        with ExitStack() as ctx:
            singles = ctx.enter_context(tc.tile_pool(name="singles", bufs=1))
            xt_pool = ctx.enter_context(tc.tile_pool(name="xt", bufs=DEPTH + 1))
            h1_pool = ctx.enter_context(tc.tile_pool(name="h1", bufs=B["h1"]))
            h2_pool = ctx.enter_context(tc.tile_pool(name="h2", bufs=B["h2"]))
            psum_s = ctx.enter_context(tc.tile_pool(name="ps", bufs=1, space="PSUM"))
            # static psum tiles; subtile dependency tracking orders reuse
            P1S = psum_s.tile([P, 2, GS * P], f32, tag="p1s")
            P2S = psum_s.tile([P, 2, GS * P], f32, tag="p2s")

            # dummy first activation: hoists the Silu table load to t=0 on the
            # otherwise-idle ACT queue (it would otherwise land right before
            # the first real silu, on the critical path)
            DUM = singles.tile([P, 1], f32)
            nc.scalar.activation(DUM[:], DUM[:], SILU)

            WB = singles.tile([P, 6160], fp8)
            # species-0 W1 chunk first on the SP queue, ahead of the xt
            # prefetches, so L1(0) starts as early as possible
            nc.sync.dma_start(WB[:, 0:1024], wb_d.ap()[:, 0:1024])
            E = singles.tile([P, T], f32)

            def w1_ap(s, c, m1):     # [p, j, m] fp8 stationary chunk
                o = ((s * 2 + c) * 2 + m1) * 256
                return WB[:, o:o + 256].rearrange("p (j m) -> p j m", j=2)

            def w2_ap(s, m1):        # [p, jj, m] fp8
                o = 4096 + (s * 2 + m1) * 256
                return WB[:, o:o + 256].rearrange("p (jj m) -> p jj m", jj=2)

            def w3_ap(s, m1):        # [p, 1] bf16
                o = 6144 + (s * 2 + m1) * 2
                return WB[:, o:o + 2].bitcast(bf16)

            xts = {}

            def dma_in(g):
                if g >= G:
                    return
                lo, hi = bounds[g]
                tl = hi - lo
                XT = xt_pool.tile([P, GS, 2, 2, P], fp8)
                if g == 0:
                    # split so the first tiles' compute starts early
                    for i, i1 in ((0, 2), (2, tl)):
                        if i1 > i:
                            nc.sync.dma_start(XT[:, i:i1],
                                              xt_d.ap()[:, lo + i:lo + i1])
                else:
                    nc.sync.dma_start(XT[:, 0:tl], xt_d.ap()[:, lo:hi])
                xts[g] = XT

            for a in range(DEPTH):
                dma_in(a)
            nc.sync.dma_start(WB[:, 1024:4096], wb_d.ap()[:, 1024:4096])
            nc.sync.dma_start(WB[:, 4096:6160], wb_d.ap()[:, 4096:6160])

            def w3_group(g2, dst):
                # per-atom energies of group g2 into a psum corner + E copy
                H2 = h2s.pop(g2)
                lo, hi2 = bounds[g2]
                tl2 = hi2 - lo
                for i in range(tl2):
                    s = species_map[lo + i]
                    for m1 in range(2):
                        nc.tensor.matmul(
                            dst[:, 1, EO + i:EO + i + 1],
                            H2[:, m1, P * i:P * (i + 1)], w3_ap(s, m1),
                            start=(m1 == 0), stop=(m1 == 1))
                nc.vector.tensor_copy(E[:, lo:lo + tl2], dst[:, 1, EO:EO + tl2])
                return lo, tl2

            def l1_tiles(g, i0, i1):
                XT = xts[g]
                for i in range(i0, i1):
                    s = species_map[bounds[g][0] + i]
                    for m1 in range(2):
                        for c in range(2):
                            nc.tensor.matmul(
                                P1S[:, m1, P * i:P * (i + 1)],
                                w1_ap(s, c, m1), XT[:, i, c],
                                start=(c == 0), stop=(c == 1),
                                perf_mode=DR)

            h1s, h2s = {}, {}
            for g in range(G + 2):
                if g < G:
                    dma_in(g + DEPTH)
                    if g == 0:
                        # interleave: deps are per-engine emission counters,
                        # so the first silu piece must be emitted before the
                        # later tiles' matmuls to start early
                        tl = bounds[0][1]
                        h = min(2, tl)
                        H1 = h1_pool.tile([P, 2, GS * P], fp8)
                        l1_tiles(0, 0, h)
                        nc.scalar.activation(H1[:, :, 0:P * h],
                                             P1S[:, :, 0:P * h],
                                             SILU, scale=scale1)
                        if tl > h:
                            l1_tiles(0, h, tl)
                            nc.scalar.activation(H1[:, :, P * h:P * tl],
                                                 P1S[:, :, P * h:P * tl],
                                                 SILU, scale=scale1)
                        h1s[0] = H1
                        xts.pop(0)
                    else:
                        l1_tiles(g, 0, bounds[g][1] - bounds[g][0])
                        xts.pop(g)
                if 2 <= g < G:
                    # W3 of the group silu2'd one ACT slot ago; e lands in a
                    # dead corner of the p2 slot (idle between silu2(g-2)'s
                    # read and L2(g-1)'s write). Its dependency (silu2(g-2))
                    # is long done, so this never delays the ACT queue.
                    w3_group(g - 2, P2S)
                if 1 <= g <= G:
                    tl = bounds[g - 1][1] - bounds[g - 1][0]
                    H1p = h1s.pop(g - 1)
                    H2 = h2_pool.tile([P, 2, GS * P], bf16)
                    # last group: write P1S (dead once s1(G-1) has read it) so
                    # L2 runs during silu2(G-2) instead of after its drain
                    PL2 = P2S if g < G else P1S
                    for i in range(tl):
                        s = species_map[bounds[g - 1][0] + i]
                        for m1 in range(2):
                            nc.tensor.matmul(
                                PL2[:, m1, P * i:P * (i + 1)],
                                w2_ap(s, m1), H1p[:, :, P * i:P * (i + 1)],
                                start=True, stop=True, perf_mode=DR)
                    nc.scalar.activation(H2[:, :, 0:P * tl],
                                         PL2[:, :, 0:P * tl],
                                         SILU, scale=scale2)
                    h2s[g - 1] = H2
                if 1 <= g < G:
                    tl = bounds[g][1] - bounds[g][0]
                    H1 = h1_pool.tile([P, 2, GS * P], fp8)
                    nc.scalar.activation(H1[:, :, 0:P * tl],
                                         P1S[:, :, 0:P * tl],
                                         SILU, scale=scale1)
                    h1s[g] = H1
                if g == G and g >= 2:
                    # P2S is dead now (last L2 went to P1S): W3(G-2)'s corner
                    # write only waits silu2(G-2), running during silu2(G-1)
                    lo, tl2 = w3_group(g - 2, P2S)
                    nc.sync.dma_start(out_d.ap()[:, 0:lo + tl2],
                                      E[:, 0:lo + tl2])
                if g == G + 1 and g >= 3:
                    lo, tl2 = w3_group(g - 2, P1S)
                    nc.sync.dma_start(out_d.ap()[:, lo:lo + tl2],
                                      E[:, lo:lo + tl2])

    nc.compile()
    return nc


# ----------------------------------------------------------------------------
# Aggregation
# ----------------------------------------------------------------------------

def aggregate(results, meta, batch, species_idx, W_comp):
    W_comp = np.asarray(W_comp, dtype=np.float64)
    batch = np.asarray(batch).astype(np.int64)
    perms = meta["perms"]
    E = np.zeros(N_STRUCT, dtype=np.float64)
    for c in range(N_CORES):
        e = np.asarray(results[c]["e_out"], dtype=np.float64)  # [128, T]
        e_slots = e.T.reshape(-1)                 # slot = t*128 + lane
        pm = perms[c]
        valid = pm >= 0
        E += np.bincount(batch[pm[valid]], weights=e_slots[valid],
                         minlength=N_STRUCT)
    energies = (E / AVG_N_ATOMS)[:, None] * E_SCALE \
        + meta["counts_ss"] @ W_comp.T
    return energies.astype(np.float32)


# ----------------------------------------------------------------------------
# Entry point
# ----------------------------------------------------------------------------

_PROGRAM_CACHE = {}


def kernel(ps, ln_gamma, ln_beta, W1, W2, W3, W_comp, species_idx, batch):
    from concourse import bass_utils

    in_maps, meta = host_prep(ps, ln_gamma, ln_beta, W1, W2, W3, W_comp,
                              species_idx, batch)
    key = (meta["T"], meta["species_map"], meta["scale1"], meta["scale2"])
    if key not in _PROGRAM_CACHE:
        _PROGRAM_CACHE[key] = build_program(meta["T"], meta["species_map"],
                                            meta["scale1"], meta["scale2"])
    nc = _PROGRAM_CACHE[key]
    res = bass_utils.run_bass_kernel_spmd(nc, in_maps,
                                          core_ids=list(range(N_CORES)))
    return aggregate(res.results, meta, batch, species_idx, W_comp)
